# revision 41
# baseline (speedup 1.0000x reference)
"""Trainium2 Bass kernel for nn_DeepConv1d (self-contained).

Math (per batch b):
  xr   = linear-interp(deep, 1024 -> 4096)           # commutes with 1x1 conv
  y    = conv_w @ xr + conv_b                        # == interp(conv_w @ deep + conv_b)
  xs   = GAMA*(y-mean)/(var_unbiased+EPS)            # per-channel over n
  loss_k[c,l] = sech^2(xs_pad[c,l+k]-xs_pad[c,l+3])  # k=0..6, reflect pad 3
  S    = sum_k loss_k ;  W_k = (loss_k/S)*x_pad[:,l+k]
  out[o,l] = sum_{c,k} fc_w[o, 7c+k] * W_k[c,l]

On-chip identities:
  - interp(conv(.)) == conv(interp(.)); interp via first differences D.
  - sech^2(d) = 4*sigmoid(2d)*sigmoid(-2d); the normalization scale
    f = GAMA/(var+EPS) folds into the sigmoid's per-partition scale
    (the mean cancels inside differences). lv' = (sigma-1)*sigma is the
    negated loss/4; fc is host-negated for k != 3 to compensate, and
    fc[:,3] host-scaled by 1/4 (W_3 = G4*x with G4 = 4/S).
  - loss_k arrays are shifted views of 3 gap arrays lv_g (g = |k-3|):
      k<3: loss_k[l] = lv_g[l+k] (g=3-k);  k>3: loss_k[l] = lv_g[l+3] (g=k-3).
  - S-sum runs on the PE: 6 accumulating matmuls with lhsT = -I plus a
    +0.25 constant, so PSUM = 0.25 + sum|lv| = S/4 and G4 = 1/PSUM.

Layout: 2 batches per core packed on 128 partitions (64 channels each).
Conv runs in bf16; sigmoid outputs are bf16 so the lv STT runs in 2x DVE
mode. GPSIMD is left idle: co-running it with DVE contends for the shared
SBUF port and slows both (measured ~2x inflation on both engines).
"""
import contextlib
import os

os.environ.setdefault("NEURON_RT_RESET_CORES", "1")

import numpy as np
import ml_dtypes

import concourse.bass as bass
import concourse.bacc as bacc_mod
import concourse.mybir as mybir
import concourse.tile as tile
from concourse.bass_utils import run_bass_kernel_spmd

bf16 = ml_dtypes.bfloat16
AF = mybir.ActivationFunctionType
ALU = mybir.AluOpType

KS = 7
PAD = 3
GAMA = 0.5
EPS = 1e-9
N = 4096
ND = 1024
NP = N + 2 * PAD       # 4102
L3 = N + PAD           # 4099: lv array length
NCORES = 8
NCH = 4                # l-chunks
CW = N // NCH          # 1024

F32 = mybir.dt.float32
BF = mybir.dt.bfloat16

USE_ACT_RECIP = False  # G4 via ACT Ln_prime (1/x) from PSUM; else DVE recip
                       # (Ln_prime is not in walrus's ACT table sets -> DVE)


def kernel_body(tc, xp_d, cph_d, dpq_d, cb_d, fck_d, nid_d, out_d):
    nc = tc.nc

    ctx = contextlib.ExitStack()
    with ctx:
        io = ctx.enter_context(tc.tile_pool(name="io", bufs=1))
        mid = ctx.enter_context(tc.tile_pool(name="mid", bufs=1))
        loss = ctx.enter_context(tc.tile_pool(name="loss", bufs=1))
        ck = ctx.enter_context(tc.tile_pool(name="ck", bufs=2))
        stp = ctx.enter_context(tc.tile_pool(name="stp", bufs=4))
        pp = ctx.enter_context(tc.tile_pool(name="pp", bufs=2, space="PSUM"))
        ppa = ctx.enter_context(tc.tile_pool(name="ppa", bufs=4, space="PSUM"))

        # ---------------- input DMAs (small first) ----------------
        cph = io.tile([64, 4, 128], BF, tag="cph")
        nc.sync.dma_start(out=cph, in_=cph_d[:, :, :])
        dpq01 = io.tile([64, ND], BF, tag="dpq01")
        nc.sync.dma_start(out=dpq01, in_=dpq_d[0:64, :])
        dpq23 = io.tile([64, ND], BF, tag="dpq23")
        nc.sync.dma_start(out=dpq23, in_=dpq_d[64:128, :])
        cb = io.tile([128, 1], F32, tag="cb")
        nc.sync.dma_start(out=cb, in_=cb_d[:, :])
        nid = io.tile([128, 128], BF, tag="nid")
        nc.sync.dma_start(out=nid, in_=nid_d[:, :])
        fck = io.tile([128, KS, 128], BF, tag="fck")
        nc.sync.dma_start(out=fck, in_=fck_d[:, :, :])
        xp = io.tile([128, NP], BF, tag="xp")          # x reflect-padded
        xs1 = io.tile([128, NP - 1], BF, tag="xs1")    # same, shifted 1 elem
        nc.sync.dma_start(out=xp, in_=xp_d[:, :])
        nc.sync.dma_start(out=xs1, in_=xp_d[:, 1:NP])
        # warm the sigmoid table with a DMA-independent input
        wz = mid.tile([128, 1], F32, tag="wz")
        nc.vector.memset(wz, 0.0)
        warm = mid.tile([128, 1], F32, tag="warm")
        nc.scalar.activation(out=warm, in_=wz, func=AF.Sigmoid, scale=1.0)

        # (-I) @ (-0.25) = +0.25 per partition: same nid lhsT as the lv
        # terms, so the S-sum accumulation never switches weights.
        q512 = io.tile([128, 512], BF, tag="q512")
        nc.vector.memset(q512, -0.25)

        # ------- conv+interp fused on the PE (phase-decomposed) ----------
        # y[4j+r] = a_r*ys[j] + b_r*ys[j+s_r]  (s=-1 for r<2, +1 for r>=2)
        # == one matmul per phase against host-stacked [dp; dp_shifted].
        # ACT applies bias and writes the phase interleaved into ypad
        # (stride-4), accumulating sum(y) per phase; a second ACT pass per
        # phase accumulates sum(y^2) from PSUM.
        ypad = mid.tile([128, NP], BF, tag="ypad")
        sdump = mid.tile([128, ND], BF, tag="sdump")
        ysum = [mid.tile([128, 1], F32, tag=f"ysum{r}", name=f"ysum{r}")
                for r in range(4)]
        sq = [mid.tile([128, 1], F32, tag=f"sq{i}", name=f"sq{i}")
              for i in range(4)]
        yc = [mid.tile([128, ND], BF, tag=f"yc{r}", name=f"yc{r}")
              for r in range(4)]
        for r in range(4):
            ypr = pp.tile([128, ND], F32, tag="ys", name=f"yph{r}")
            dq = dpq01 if r < 2 else dpq23
            for h in range(2):
                nc.tensor.matmul(
                    out=ypr[:, h * 512:(h + 1) * 512],
                    lhsT=cph[:, r, :],
                    rhs=dq[:, h * 512:(h + 1) * 512],
                    start=True, stop=True,
                )
            # contiguous ACT write (a stride-4 ACT write runs ~2.4x slower);
            # the interleave into ypad and the sum(y^2) reduce run on the
            # DVE, which is idle during this window, keeping the serial ACT
            # chain to just the 4 Identity copies.
            nc.scalar.activation(out=yc[r], in_=ypr, func=AF.Identity, bias=cb,
                                 scale=1.0, accum_out=ysum[r])
            dst = bass.AP(tensor=ypad.tensor, offset=ypad.offset + PAD + r,
                          ap=[list(ypad.ap[0]), [4, ND]])
            nc.vector.tensor_copy(out=dst, in_=yc[r])
        # squares after all copies: they overlap the dy subs instead of
        # delaying the last Identity (and hence dy) on the serial ACT queue
        for r in range(4):
            nc.scalar.activation(out=sdump, in_=yc[r], func=AF.Square,
                                 accum_out=sq[r])
        # reflect edges: ypad[2-i] = ypad[4+i], ypad[N+3+i] = ypad[N+1-i]
        for i in range(3):
            nc.vector.tensor_copy(out=ypad[:, 2 - i:3 - i], in_=ypad[:, 4 + i:5 + i])
            nc.vector.tensor_copy(
                out=ypad[:, N + 3 + i:N + 4 + i], in_=ypad[:, N + 1 - i:N + 2 - i])

        # ---------------- gap diffs (bf16, DVE 2x) ----------------
        # dy3 first (its sigmoid is first on the ACT queue); the tiny stats
        # chain below is emitted before dy2b/dy1 so f2p is not queued behind
        # them on the DVE.
        dy1 = loss.tile([128, L3], BF, tag="T1")
        dy2b = loss.tile([128, L3], BF, tag="T2")
        dy3 = loss.tile([128, L3], BF, tag="T3")
        nc.vector.tensor_sub(out=dy3, in0=ypad[:, 3:3 + L3], in1=ypad[:, 0:L3])

        # ---------------- stats -> sigmoid scale ----------------
        ts0 = mid.tile([128, 1], F32, tag="ts0")
        nc.vector.tensor_add(out=ts0, in0=ysum[0], in1=ysum[1])
        ts1 = mid.tile([128, 1], F32, tag="ts1")
        nc.vector.tensor_add(out=ts1, in0=ysum[2], in1=ysum[3])
        sum_y = mid.tile([128, 1], F32, tag="sum_y")
        nc.vector.tensor_add(out=sum_y, in0=ts0, in1=ts1)
        tc_ = mid.tile([128, 1], F32, tag="tc_")
        nc.vector.tensor_add(out=tc_, in0=sq[0], in1=sq[1])
        td = mid.tile([128, 1], F32, tag="td")
        nc.vector.tensor_add(out=td, in0=sq[2], in1=sq[3])
        sum_y2 = mid.tile([128, 1], F32, tag="sum_y2")
        nc.vector.tensor_add(out=sum_y2, in0=tc_, in1=td)
        # mean = sum_y/N; var = (sum_y2 - sum_y*mean)/(N-1); f = GAMA/(var+EPS)
        mean = mid.tile([128, 1], F32, tag="mean")
        nc.vector.tensor_scalar_mul(out=mean, in0=sum_y, scalar1=1.0 / N)
        t0 = mid.tile([128, 1], F32, tag="t0")
        nc.vector.tensor_mul(out=t0, in0=sum_y, in1=mean)
        t2 = mid.tile([128, 1], F32, tag="t2")
        nc.vector.tensor_sub(out=t2, in0=sum_y2, in1=t0)
        denom = mid.tile([128, 1], F32, tag="denom")
        nc.vector.tensor_scalar(out=denom, in0=t2, scalar1=1.0 / (N - 1),
                                scalar2=EPS, op0=ALU.mult, op1=ALU.add)
        inv = mid.tile([128, 1], F32, tag="inv")
        nc.vector.reciprocal(out=inv, in_=denom)
        f2p = mid.tile([128, 1], F32, tag="f2p")
        nc.vector.tensor_scalar_mul(out=f2p, in0=inv, scalar1=2.0 * GAMA)
        nc.vector.tensor_sub(out=dy2b, in0=ypad[:, 3:3 + L3], in1=ypad[:, 1:1 + L3])
        nc.vector.tensor_sub(out=dy1, in0=ypad[:, 1:1 + L3], in1=ypad[:, 0:L3])

        # ---------------- sigmoids (ACT, bf16 out) + lv' = (sa-1)*sa ------
        # The STT only has a 1x DVE uop, but keeping 3 sigmoids (not 6 of a
        # +- pair) keeps the serial ACT chain off the critical path; the STT
        # overlaps the next sigmoid. lv' is negated; fc is host-negated for
        # k != 3 to compensate.
        # Halved sigmoid+STT: the first lv halves land while later sigmoids
        # still run, so the S-sum matmuls and chunk 0/1 start earlier.
        # lv gets its OWN buffers (L* tags): aliasing dy's buffer makes the
        # first-half STT wait for the second-half sigmoid's read (the WAR
        # dependency is tile-granular, not range-granular).
        HH = 2052  # first-half width (even, >= CW*2+4 so chunks 0,1 covered)
        sa3 = loss.tile([128, L3], BF, tag="S4")
        sa2 = loss.tile([128, L3], BF, tag="S2")
        sa1 = loss.tile([128, L3], BF, tag="S0")
        lv3 = loss.tile([128, L3], BF, tag="L3")
        lv2b = loss.tile([128, L3], BF, tag="L2")
        lv1 = loss.tile([128, L3], BF, tag="L1")
        gaps = ((sa3, dy3, lv3), (sa2, dy2b, lv2b), (sa1, dy1, lv1))
        SL_A, SL_B = slice(0, HH), slice(HH, L3)
        for sa, dy, lv in gaps:
            nc.scalar.activation(out=sa[:, SL_A], in_=dy[:, SL_A],
                                 func=AF.Sigmoid, scale=f2p)
            nc.vector.scalar_tensor_tensor(
                out=lv[:, SL_A], in0=sa[:, SL_A], scalar=1.0, in1=sa[:, SL_A],
                op0=ALU.subtract, op1=ALU.mult)
        for sa, dy, lv in gaps:
            nc.scalar.activation(out=sa[:, SL_B], in_=dy[:, SL_B],
                                 func=AF.Sigmoid, scale=f2p)

        # S-sum terms: msum = 0.25 + sum|lv| accumulated on the PE
        terms = [(lv1, 2), (lv1, 3), (lv2b, 0), (lv2b, 2), (lv3, 0), (lv3, 3)]
        W_of = {}

        def emit_front(c):
            """msum (PE) -> G4 (DVE recip + ACT cast) -> P/GL/W (DVE)."""
            lo = c * CW
            msum_ps = pp.tile([128, CW], F32, tag="ys", name=f"msum{c}")
            for h in range(2):
                base = lo + h * 512
                sub = msum_ps[:, h * 512:(h + 1) * 512]
                nc.tensor.matmul(out=sub, lhsT=nid, rhs=q512,
                                 start=True, stop=False)
                for t, (arr, off) in enumerate(terms):
                    nc.tensor.matmul(
                        out=sub, lhsT=nid,
                        rhs=arr[:, base + off:base + off + 512],
                        start=False, stop=(t == 5),
                    )
            Pc0 = ck.tile([128, CW], BF, tag="P0", name=f"P0_{c}")
            Pc1 = ck.tile([128, CW], BF, tag="P1", name=f"P1_{c}")
            Pc2 = ck.tile([128, CW], BF, tag="P2", name=f"P2_{c}")
            nc.vector.tensor_mul(out=Pc0, in0=lv3[:, lo:lo + CW],
                                 in1=xp[:, lo:lo + CW])
            nc.vector.tensor_mul(out=Pc1, in0=lv2b[:, lo:lo + CW],
                                 in1=xs1[:, lo:lo + CW])
            nc.vector.tensor_mul(out=Pc2, in0=lv1[:, lo + 2:lo + 2 + CW],
                                 in1=xp[:, lo + 2:lo + 2 + CW])
            G4 = ck.tile([128, CW], BF, tag="G4", name=f"G4_{c}")
            G32 = ck.tile([128, CW], F32, tag="G32", name=f"G32_{c}")
            nc.vector.reciprocal_approx_fast(out=G32, in_=msum_ps)
            nc.scalar.copy(out=G4, in_=G32)

            GL1 = ck.tile([128, CW], BF, tag="GL1", name=f"GL1_{c}")
            GL2 = ck.tile([128, CW], BF, tag="GL2", name=f"GL2_{c}")
            GL3 = ck.tile([128, CW], BF, tag="GL3", name=f"GL3_{c}")
            nc.vector.tensor_mul(out=GL1, in0=lv1[:, lo + 3:lo + 3 + CW], in1=G4)
            nc.vector.tensor_mul(out=GL2, in0=lv2b[:, lo + 2:lo + 2 + CW], in1=G4)
            nc.vector.tensor_mul(out=GL3, in0=lv3[:, lo + 3:lo + 3 + CW], in1=G4)

            W = [ck.tile([128, CW], BF, tag=f"W{k}", name=f"W{k}_{c}")
                 for k in range(KS)]
            nc.vector.tensor_mul(out=W[0], in0=G4, in1=Pc0)
            nc.vector.tensor_mul(out=W[1], in0=G4, in1=Pc1)
            nc.vector.tensor_mul(out=W[2], in0=G4, in1=Pc2)
            nc.vector.tensor_mul(out=W[3], in0=G4, in1=xs1[:, lo + 2:lo + 2 + CW])
            nc.vector.tensor_mul(out=W[4], in0=GL1, in1=xp[:, lo + 4:lo + 4 + CW])
            nc.vector.tensor_mul(out=W[5], in0=GL2, in1=xs1[:, lo + 4:lo + 4 + CW])
            nc.vector.tensor_mul(out=W[6], in0=GL3, in1=xp[:, lo + 6:lo + 6 + CW])
            W_of[c] = W

        def emit_back(c):
            """GEMM (PE) -> obuf copies (ACT) -> chunk-contiguous DMA."""
            lo = c * CW
            W = W_of[c]
            obuf = stp.tile([128, 2, CW], BF, tag="obuf", name=f"obuf_{c}")
            for b in range(2):
                prow = slice(64 * b, 64 * (b + 1))
                for sub_i in range(CW // 512):
                    acc = ppa.tile([128, 512], F32, tag="acc",
                                   name=f"acc_{c}_{b}_{sub_i}")
                    cs = slice(sub_i * 512, (sub_i + 1) * 512)
                    for k in range(KS):
                        nc.tensor.matmul(
                            out=acc,
                            lhsT=fck[prow, k, :],
                            rhs=W[k][prow, cs],
                            start=(k == 0), stop=(k == KS - 1),
                        )
                    # last chunk: split copies DVE/ACT so the tail runs them
                    # in parallel (DVE is already drained by then)
                    if c == NCH - 1 and sub_i == 1:
                        nc.vector.tensor_copy(out=obuf[:, b, cs], in_=acc)
                    else:
                        nc.scalar.copy(out=obuf[:, b, cs], in_=acc)
            nc.sync.dma_start(out=out_d[:, c, :, :], in_=obuf)

        # Emission order: chunks 0/1 (covered by the first lv halves) are
        # interleaved with the second sigmoid/STT halves so neither the DVE
        # nor the ACT queue blocks on data that is not ready yet.
        emit_front(0)
        emit_front(1)
        emit_back(0)
        for sa, dy, lv in gaps:
            nc.vector.scalar_tensor_tensor(
                out=lv[:, SL_B], in0=sa[:, SL_B], scalar=1.0, in1=sa[:, SL_B],
                op0=ALU.subtract, op1=ALU.mult)
        emit_back(1)
        emit_front(2)
        emit_back(2)
        emit_front(3)
        emit_back(3)


def build_nc():
    nc = bacc_mod.Bacc(None, target_bir_lowering=False)
    xp_d = nc.dram_tensor("xp", [128, NP], BF, kind="ExternalInput")
    cph_d = nc.dram_tensor("cph", [64, 4, 128], BF, kind="ExternalInput")
    dpq_d = nc.dram_tensor("dpq", [128, ND], BF, kind="ExternalInput")
    cb_d = nc.dram_tensor("cb", [128, 1], F32, kind="ExternalInput")
    fck_d = nc.dram_tensor("fck", [128, KS, 128], BF, kind="ExternalInput")
    nid_d = nc.dram_tensor("nid", [128, 128], BF, kind="ExternalInput")
    out_d = nc.dram_tensor("out", [128, NCH, 2, CW], BF, kind="ExternalOutput")
    with tile.TileContext(nc) as tc:
        kernel_body(tc, xp_d, cph_d, dpq_d, cb_d, fck_d, nid_d, out_d)
    nc.compile()
    return nc


def prep_inputs(deep, x, conv_w, conv_b, fc_w):
    deep = np.asarray(deep, np.float32)
    x = np.asarray(x, np.float32)
    conv_w = np.asarray(conv_w, np.float32)
    conv_b = np.asarray(conv_b, np.float32)
    fc_w = np.asarray(fc_w, np.float32)

    xpad = np.pad(x, ((0, 0), (0, 0), (PAD, PAD)), mode="reflect")
    xp_all = np.ascontiguousarray(xpad.reshape(NCORES, 128, NP)).astype(bf16)
    dp_all = np.ascontiguousarray(deep.reshape(NCORES, 32, ND))
    # phase-fused conv+interp weights: y[4j+r] = a_r*ys[j] + b_r*ys[j+s_r]
    a_ph = [0.625, 0.875, 0.875, 0.625]
    b_ph = [0.375, 0.125, 0.125, 0.375]
    cwT = conv_w.T  # (16, 64)
    cph = np.zeros((64, 4, 128), np.float32)
    for r in range(4):
        cph[0:16, r, 0:64] = a_ph[r] * cwT
        cph[16:32, r, 64:128] = a_ph[r] * cwT
        cph[32:48, r, 0:64] = b_ph[r] * cwT
        cph[48:64, r, 64:128] = b_ph[r] * cwT
    cph = np.ascontiguousarray(cph).astype(bf16)
    cb = np.ascontiguousarray(
        np.concatenate([conv_b, conv_b]).reshape(128, 1).astype(np.float32))
    nid = (-np.eye(128)).astype(bf16)
    fc3 = fc_w.reshape(128, 64, KS)
    fck_half = np.transpose(fc3, (1, 2, 0)).copy()
    fck_half *= -1.0              # lv' is computed negated on-chip
    fck_half[:, PAD, :] *= -0.25  # W_3 = G4*x = 4*(G*x), not lv-scaled
    fck = np.ascontiguousarray(
        np.concatenate([fck_half, fck_half], axis=0)).astype(bf16)
    maps = []
    for ci in range(NCORES):
        dp2 = dp_all[ci]                                  # (32, ND) b0;b1
        dpm = np.concatenate([dp2[:, :1], dp2[:, :-1]], axis=1)   # dp[j-1]
        dpp = np.concatenate([dp2[:, 1:], dp2[:, -1:]], axis=1)   # dp[j+1]
        dpq = np.ascontiguousarray(
            np.concatenate([dp2, dpm, dp2, dpp], axis=0)).astype(bf16)
        maps.append({"xp": np.ascontiguousarray(xp_all[ci]),
                     "cph": cph, "dpq": dpq,
                     "cb": cb, "fck": fck, "nid": nid})
    return maps


def gather_out(results):
    out_full = np.empty((16, 128, N), np.float32)
    for ci in range(NCORES):
        o = np.asarray(results[ci]["out"], dtype=np.float32)
        o = np.transpose(o, (0, 2, 1, 3)).reshape(128, 2, N)
        out_full[2 * ci] = o[:, 0]
        out_full[2 * ci + 1] = o[:, 1]
    return out_full


_CACHED = {}


def _get_nc():
    if "nc" not in _CACHED:
        _CACHED["nc"] = build_nc()
    return _CACHED["nc"]


def kernel(deep, x, conv_w, conv_b, fc_w):
    in_maps = prep_inputs(deep, x, conv_w, conv_b, fc_w)
    nc = _get_nc()
    res = run_bass_kernel_spmd(nc, in_maps, core_ids=list(range(NCORES)))
    return gather_out(res.results)


# revision 42
# speedup vs baseline: 1.2343x; 1.2343x over previous
"""Trainium2 Bass kernel for nn_DeepConv1d (self-contained).

Math (per batch b):
  xr   = linear-interp(deep, 1024 -> 4096)           # commutes with 1x1 conv
  y    = conv_w @ xr + conv_b                        # == interp(conv_w @ deep + conv_b)
  xs   = GAMA*(y-mean)/(var_unbiased+EPS)            # per-channel over n
  loss_k[c,l] = sech^2(xs_pad[c,l+k]-xs_pad[c,l+3])  # k=0..6, reflect pad 3
  S    = sum_k loss_k ;  W_k = (loss_k/S)*x_pad[:,l+k]
  out[o,l] = sum_{c,k} fc_w[o, 7c+k] * W_k[c,l]

On-chip identities:
  - interp(conv(.)) == conv(interp(.)); interp via first differences D.
  - sech^2(d) = 4*sigmoid(2d)*sigmoid(-2d); the normalization scale
    f = GAMA/(var+EPS) folds into the sigmoid's per-partition scale
    (the mean cancels inside differences). lv' = (sigma-1)*sigma is the
    negated loss/4; fc is host-negated for k != 3 to compensate, and
    fc[:,3] host-scaled by 1/4 (W_3 = G4*x with G4 = 4/S).
  - loss_k arrays are shifted views of 3 gap arrays lv_g (g = |k-3|):
      k<3: loss_k[l] = lv_g[l+k] (g=3-k);  k>3: loss_k[l] = lv_g[l+3] (g=k-3).
  - S-sum runs on the PE: 6 accumulating matmuls with lhsT = -I plus a
    +0.25 constant, so PSUM = 0.25 + sum|lv| = S/4 and G4 = 1/PSUM.

Layout: 2 batches per core packed on 128 partitions (64 channels each).
Conv runs in bf16; sigmoid outputs are bf16 so the lv STT runs in 2x DVE
mode. GPSIMD is left idle: co-running it with DVE contends for the shared
SBUF port and slows both (measured ~2x inflation on both engines).
"""
import contextlib

import numpy as np
import ml_dtypes

import concourse.bass as bass
import concourse.bacc as bacc_mod
import concourse.mybir as mybir
import concourse.tile as tile
from concourse.bass_utils import run_bass_kernel_spmd

bf16 = ml_dtypes.bfloat16
AF = mybir.ActivationFunctionType
ALU = mybir.AluOpType

KS = 7
PAD = 3
GAMA = 0.5
EPS = 1e-9
N = 4096
ND = 1024
NP = N + 2 * PAD       # 4102
L3 = N + PAD           # 4099: lv array length
NCORES = 8
NCH = 4                # l-chunks
CW = N // NCH          # 1024

F32 = mybir.dt.float32
BF = mybir.dt.bfloat16

USE_ACT_RECIP = False  # G4 via ACT Ln_prime (1/x) from PSUM; else DVE recip
                       # (Ln_prime is not in walrus's ACT table sets -> DVE)


def kernel_body(tc, xp_d, cph_d, dpq_d, cb_d, fck_d, nid_d, out_d):
    nc = tc.nc

    ctx = contextlib.ExitStack()
    with ctx:
        io = ctx.enter_context(tc.tile_pool(name="io", bufs=1))
        mid = ctx.enter_context(tc.tile_pool(name="mid", bufs=1))
        loss = ctx.enter_context(tc.tile_pool(name="loss", bufs=1))
        ck = ctx.enter_context(tc.tile_pool(name="ck", bufs=2))
        stp = ctx.enter_context(tc.tile_pool(name="stp", bufs=4))
        pp = ctx.enter_context(tc.tile_pool(name="pp", bufs=2, space="PSUM"))
        ppa = ctx.enter_context(tc.tile_pool(name="ppa", bufs=4, space="PSUM"))

        # ---------------- input DMAs (small first) ----------------
        cph = io.tile([64, 4, 128], BF, tag="cph")
        nc.sync.dma_start(out=cph, in_=cph_d[:, :, :])
        dpq01 = io.tile([64, ND], BF, tag="dpq01")
        nc.sync.dma_start(out=dpq01, in_=dpq_d[0:64, :])
        dpq23 = io.tile([64, ND], BF, tag="dpq23")
        nc.sync.dma_start(out=dpq23, in_=dpq_d[64:128, :])
        cb = io.tile([128, 1], F32, tag="cb")
        nc.sync.dma_start(out=cb, in_=cb_d[:, :])
        nid = io.tile([128, 128], BF, tag="nid")
        nc.sync.dma_start(out=nid, in_=nid_d[:, :])
        fck = io.tile([128, KS, 128], BF, tag="fck")
        nc.sync.dma_start(out=fck, in_=fck_d[:, :, :])
        xp = io.tile([128, NP], BF, tag="xp")          # x reflect-padded
        xs1 = io.tile([128, NP - 1], BF, tag="xs1")    # same, shifted 1 elem
        nc.sync.dma_start(out=xp, in_=xp_d[:, :])
        nc.sync.dma_start(out=xs1, in_=xp_d[:, 1:NP])
        # warm the sigmoid table with a DMA-independent input
        wz = mid.tile([128, 1], F32, tag="wz")
        nc.vector.memset(wz, 0.0)
        warm = mid.tile([128, 1], F32, tag="warm")
        nc.scalar.activation(out=warm, in_=wz, func=AF.Sigmoid, scale=1.0)

        # (-I) @ (-0.25) = +0.25 per partition: same nid lhsT as the lv
        # terms, so the S-sum accumulation never switches weights.
        q512 = io.tile([128, 512], BF, tag="q512")
        nc.vector.memset(q512, -0.25)

        # ------- conv+interp fused on the PE (phase-decomposed) ----------
        # y[4j+r] = a_r*ys[j] + b_r*ys[j+s_r]  (s=-1 for r<2, +1 for r>=2)
        # == one matmul per phase against host-stacked [dp; dp_shifted].
        # ACT applies bias and writes the phase interleaved into ypad
        # (stride-4), accumulating sum(y) per phase; a second ACT pass per
        # phase accumulates sum(y^2) from PSUM.
        ypad = mid.tile([128, NP], BF, tag="ypad")
        sdump = mid.tile([128, ND], BF, tag="sdump")
        ysum = [mid.tile([128, 1], F32, tag=f"ysum{r}", name=f"ysum{r}")
                for r in range(4)]
        sq = [mid.tile([128, 1], F32, tag=f"sq{i}", name=f"sq{i}")
              for i in range(4)]
        yc = [mid.tile([128, ND], BF, tag=f"yc{r}", name=f"yc{r}")
              for r in range(4)]
        for r in range(4):
            ypr = pp.tile([128, ND], F32, tag="ys", name=f"yph{r}")
            dq = dpq01 if r < 2 else dpq23
            for h in range(2):
                nc.tensor.matmul(
                    out=ypr[:, h * 512:(h + 1) * 512],
                    lhsT=cph[:, r, :],
                    rhs=dq[:, h * 512:(h + 1) * 512],
                    start=True, stop=True,
                )
            # contiguous ACT write (a stride-4 ACT write runs ~2.4x slower);
            # the interleave into ypad and the sum(y^2) reduce run on the
            # DVE, which is idle during this window, keeping the serial ACT
            # chain to just the 4 Identity copies.
            nc.scalar.activation(out=yc[r], in_=ypr, func=AF.Identity, bias=cb,
                                 scale=1.0, accum_out=ysum[r])
            dst = bass.AP(tensor=ypad.tensor, offset=ypad.offset + PAD + r,
                          ap=[list(ypad.ap[0]), [4, ND]])
            nc.vector.tensor_copy(out=dst, in_=yc[r])
        # squares after all copies: they overlap the dy subs instead of
        # delaying the last Identity (and hence dy) on the serial ACT queue
        for r in range(4):
            nc.scalar.activation(out=sdump, in_=yc[r], func=AF.Square,
                                 accum_out=sq[r])
        # reflect edges: ypad[2-i] = ypad[4+i], ypad[N+3+i] = ypad[N+1-i]
        for i in range(3):
            nc.vector.tensor_copy(out=ypad[:, 2 - i:3 - i], in_=ypad[:, 4 + i:5 + i])
            nc.vector.tensor_copy(
                out=ypad[:, N + 3 + i:N + 4 + i], in_=ypad[:, N + 1 - i:N + 2 - i])

        # ---------------- gap diffs (bf16, DVE 2x) ----------------
        # dy3 first (its sigmoid is first on the ACT queue); the tiny stats
        # chain below is emitted before dy2b/dy1 so f2p is not queued behind
        # them on the DVE.
        dy1 = loss.tile([128, L3], BF, tag="T1")
        dy2b = loss.tile([128, L3], BF, tag="T2")
        dy3 = loss.tile([128, L3], BF, tag="T3")
        nc.vector.tensor_sub(out=dy3, in0=ypad[:, 3:3 + L3], in1=ypad[:, 0:L3])

        # ---------------- stats -> sigmoid scale ----------------
        ts0 = mid.tile([128, 1], F32, tag="ts0")
        nc.vector.tensor_add(out=ts0, in0=ysum[0], in1=ysum[1])
        ts1 = mid.tile([128, 1], F32, tag="ts1")
        nc.vector.tensor_add(out=ts1, in0=ysum[2], in1=ysum[3])
        sum_y = mid.tile([128, 1], F32, tag="sum_y")
        nc.vector.tensor_add(out=sum_y, in0=ts0, in1=ts1)
        tc_ = mid.tile([128, 1], F32, tag="tc_")
        nc.vector.tensor_add(out=tc_, in0=sq[0], in1=sq[1])
        td = mid.tile([128, 1], F32, tag="td")
        nc.vector.tensor_add(out=td, in0=sq[2], in1=sq[3])
        sum_y2 = mid.tile([128, 1], F32, tag="sum_y2")
        nc.vector.tensor_add(out=sum_y2, in0=tc_, in1=td)
        # mean = sum_y/N; var = (sum_y2 - sum_y*mean)/(N-1); f = GAMA/(var+EPS)
        mean = mid.tile([128, 1], F32, tag="mean")
        nc.vector.tensor_scalar_mul(out=mean, in0=sum_y, scalar1=1.0 / N)
        t0 = mid.tile([128, 1], F32, tag="t0")
        nc.vector.tensor_mul(out=t0, in0=sum_y, in1=mean)
        t2 = mid.tile([128, 1], F32, tag="t2")
        nc.vector.tensor_sub(out=t2, in0=sum_y2, in1=t0)
        denom = mid.tile([128, 1], F32, tag="denom")
        nc.vector.tensor_scalar(out=denom, in0=t2, scalar1=1.0 / (N - 1),
                                scalar2=EPS, op0=ALU.mult, op1=ALU.add)
        inv = mid.tile([128, 1], F32, tag="inv")
        nc.vector.reciprocal(out=inv, in_=denom)
        f2p = mid.tile([128, 1], F32, tag="f2p")
        nc.vector.tensor_scalar_mul(out=f2p, in0=inv, scalar1=2.0 * GAMA)
        nc.vector.tensor_sub(out=dy2b, in0=ypad[:, 3:3 + L3], in1=ypad[:, 1:1 + L3])
        nc.vector.tensor_sub(out=dy1, in0=ypad[:, 1:1 + L3], in1=ypad[:, 0:L3])

        # ---------------- sigmoids (ACT, bf16 out) + lv' = (sa-1)*sa ------
        # The STT only has a 1x DVE uop, but keeping 3 sigmoids (not 6 of a
        # +- pair) keeps the serial ACT chain off the critical path; the STT
        # overlaps the next sigmoid. lv' is negated; fc is host-negated for
        # k != 3 to compensate.
        # Halved sigmoid+STT: the first lv halves land while later sigmoids
        # still run, so the S-sum matmuls and chunk 0/1 start earlier.
        # lv gets its OWN buffers (L* tags): aliasing dy's buffer makes the
        # first-half STT wait for the second-half sigmoid's read (the WAR
        # dependency is tile-granular, not range-granular).
        HH = 2052  # first-half width (even, >= CW*2+4 so chunks 0,1 covered)
        sa3 = loss.tile([128, L3], BF, tag="S4")
        sa2 = loss.tile([128, L3], BF, tag="S2")
        sa1 = loss.tile([128, L3], BF, tag="S0")
        lv3 = loss.tile([128, L3], BF, tag="L3")
        lv2b = loss.tile([128, L3], BF, tag="L2")
        lv1 = loss.tile([128, L3], BF, tag="L1")
        gaps = ((sa3, dy3, lv3), (sa2, dy2b, lv2b), (sa1, dy1, lv1))
        SL_A, SL_B = slice(0, HH), slice(HH, L3)
        for sa, dy, lv in gaps:
            nc.scalar.activation(out=sa[:, SL_A], in_=dy[:, SL_A],
                                 func=AF.Sigmoid, scale=f2p)
            nc.vector.scalar_tensor_tensor(
                out=lv[:, SL_A], in0=sa[:, SL_A], scalar=1.0, in1=sa[:, SL_A],
                op0=ALU.subtract, op1=ALU.mult)
        for sa, dy, lv in gaps:
            nc.scalar.activation(out=sa[:, SL_B], in_=dy[:, SL_B],
                                 func=AF.Sigmoid, scale=f2p)

        # S-sum terms: msum = 0.25 + sum|lv| accumulated on the PE
        terms = [(lv1, 2), (lv1, 3), (lv2b, 0), (lv2b, 2), (lv3, 0), (lv3, 3)]
        W_of = {}

        def emit_front(c):
            """msum (PE) -> G4 (DVE recip + ACT cast) -> P/GL/W (DVE)."""
            lo = c * CW
            msum_ps = pp.tile([128, CW], F32, tag="ys", name=f"msum{c}")
            for h in range(2):
                base = lo + h * 512
                sub = msum_ps[:, h * 512:(h + 1) * 512]
                nc.tensor.matmul(out=sub, lhsT=nid, rhs=q512,
                                 start=True, stop=False)
                for t, (arr, off) in enumerate(terms):
                    nc.tensor.matmul(
                        out=sub, lhsT=nid,
                        rhs=arr[:, base + off:base + off + 512],
                        start=False, stop=(t == 5),
                    )
            Pc0 = ck.tile([128, CW], BF, tag="P0", name=f"P0_{c}")
            Pc1 = ck.tile([128, CW], BF, tag="P1", name=f"P1_{c}")
            Pc2 = ck.tile([128, CW], BF, tag="P2", name=f"P2_{c}")
            nc.vector.tensor_mul(out=Pc0, in0=lv3[:, lo:lo + CW],
                                 in1=xp[:, lo:lo + CW])
            nc.vector.tensor_mul(out=Pc1, in0=lv2b[:, lo:lo + CW],
                                 in1=xs1[:, lo:lo + CW])
            nc.vector.tensor_mul(out=Pc2, in0=lv1[:, lo + 2:lo + 2 + CW],
                                 in1=xp[:, lo + 2:lo + 2 + CW])
            G4 = ck.tile([128, CW], BF, tag="G4", name=f"G4_{c}")
            G32 = ck.tile([128, CW], F32, tag="G32", name=f"G32_{c}")
            nc.vector.reciprocal_approx_fast(out=G32, in_=msum_ps)
            nc.scalar.copy(out=G4, in_=G32)

            GL1 = ck.tile([128, CW], BF, tag="GL1", name=f"GL1_{c}")
            GL2 = ck.tile([128, CW], BF, tag="GL2", name=f"GL2_{c}")
            GL3 = ck.tile([128, CW], BF, tag="GL3", name=f"GL3_{c}")
            nc.vector.tensor_mul(out=GL1, in0=lv1[:, lo + 3:lo + 3 + CW], in1=G4)
            nc.vector.tensor_mul(out=GL2, in0=lv2b[:, lo + 2:lo + 2 + CW], in1=G4)
            nc.vector.tensor_mul(out=GL3, in0=lv3[:, lo + 3:lo + 3 + CW], in1=G4)

            W = [ck.tile([128, CW], BF, tag=f"W{k}", name=f"W{k}_{c}")
                 for k in range(KS)]
            nc.vector.tensor_mul(out=W[0], in0=G4, in1=Pc0)
            nc.vector.tensor_mul(out=W[1], in0=G4, in1=Pc1)
            nc.vector.tensor_mul(out=W[2], in0=G4, in1=Pc2)
            nc.vector.tensor_mul(out=W[3], in0=G4, in1=xs1[:, lo + 2:lo + 2 + CW])
            nc.vector.tensor_mul(out=W[4], in0=GL1, in1=xp[:, lo + 4:lo + 4 + CW])
            nc.vector.tensor_mul(out=W[5], in0=GL2, in1=xs1[:, lo + 4:lo + 4 + CW])
            nc.vector.tensor_mul(out=W[6], in0=GL3, in1=xp[:, lo + 6:lo + 6 + CW])
            W_of[c] = W

        def emit_back(c):
            """GEMM (PE) -> obuf copies (ACT) -> chunk-contiguous DMA."""
            lo = c * CW
            W = W_of[c]
            obuf = stp.tile([128, 2, CW], BF, tag="obuf", name=f"obuf_{c}")
            for b in range(2):
                prow = slice(64 * b, 64 * (b + 1))
                for sub_i in range(CW // 512):
                    acc = ppa.tile([128, 512], F32, tag="acc",
                                   name=f"acc_{c}_{b}_{sub_i}")
                    cs = slice(sub_i * 512, (sub_i + 1) * 512)
                    for k in range(KS):
                        nc.tensor.matmul(
                            out=acc,
                            lhsT=fck[prow, k, :],
                            rhs=W[k][prow, cs],
                            start=(k == 0), stop=(k == KS - 1),
                        )
                    # last chunk: split copies DVE/ACT so the tail runs them
                    # in parallel (DVE is already drained by then)
                    if c == NCH - 1 and sub_i == 1:
                        nc.vector.tensor_copy(out=obuf[:, b, cs], in_=acc)
                    else:
                        nc.scalar.copy(out=obuf[:, b, cs], in_=acc)
            nc.sync.dma_start(out=out_d[:, c, :, :], in_=obuf)

        # Emission order: chunks 0/1 (covered by the first lv halves) are
        # interleaved with the second sigmoid/STT halves so neither the DVE
        # nor the ACT queue blocks on data that is not ready yet.
        emit_front(0)
        emit_front(1)
        emit_back(0)
        for sa, dy, lv in gaps:
            nc.vector.scalar_tensor_tensor(
                out=lv[:, SL_B], in0=sa[:, SL_B], scalar=1.0, in1=sa[:, SL_B],
                op0=ALU.subtract, op1=ALU.mult)
        emit_back(1)
        emit_front(2)
        emit_back(2)
        emit_front(3)
        emit_back(3)


def build_nc():
    nc = bacc_mod.Bacc(None, target_bir_lowering=False)
    xp_d = nc.dram_tensor("xp", [128, NP], BF, kind="ExternalInput")
    cph_d = nc.dram_tensor("cph", [64, 4, 128], BF, kind="ExternalInput")
    dpq_d = nc.dram_tensor("dpq", [128, ND], BF, kind="ExternalInput")
    cb_d = nc.dram_tensor("cb", [128, 1], F32, kind="ExternalInput")
    fck_d = nc.dram_tensor("fck", [128, KS, 128], BF, kind="ExternalInput")
    nid_d = nc.dram_tensor("nid", [128, 128], BF, kind="ExternalInput")
    out_d = nc.dram_tensor("out", [128, NCH, 2, CW], BF, kind="ExternalOutput")
    with tile.TileContext(nc) as tc:
        kernel_body(tc, xp_d, cph_d, dpq_d, cb_d, fck_d, nid_d, out_d)
    nc.compile()
    return nc


def prep_inputs(deep, x, conv_w, conv_b, fc_w):
    deep = np.asarray(deep, np.float32)
    x = np.asarray(x, np.float32)
    conv_w = np.asarray(conv_w, np.float32)
    conv_b = np.asarray(conv_b, np.float32)
    fc_w = np.asarray(fc_w, np.float32)

    xpad = np.pad(x, ((0, 0), (0, 0), (PAD, PAD)), mode="reflect")
    xp_all = np.ascontiguousarray(xpad.reshape(NCORES, 128, NP)).astype(bf16)
    dp_all = np.ascontiguousarray(deep.reshape(NCORES, 32, ND))
    # phase-fused conv+interp weights: y[4j+r] = a_r*ys[j] + b_r*ys[j+s_r]
    a_ph = [0.625, 0.875, 0.875, 0.625]
    b_ph = [0.375, 0.125, 0.125, 0.375]
    cwT = conv_w.T  # (16, 64)
    cph = np.zeros((64, 4, 128), np.float32)
    for r in range(4):
        cph[0:16, r, 0:64] = a_ph[r] * cwT
        cph[16:32, r, 64:128] = a_ph[r] * cwT
        cph[32:48, r, 0:64] = b_ph[r] * cwT
        cph[48:64, r, 64:128] = b_ph[r] * cwT
    cph = np.ascontiguousarray(cph).astype(bf16)
    cb = np.ascontiguousarray(
        np.concatenate([conv_b, conv_b]).reshape(128, 1).astype(np.float32))
    nid = (-np.eye(128)).astype(bf16)
    fc3 = fc_w.reshape(128, 64, KS)
    fck_half = np.transpose(fc3, (1, 2, 0)).copy()
    fck_half *= -1.0              # lv' is computed negated on-chip
    fck_half[:, PAD, :] *= -0.25  # W_3 = G4*x = 4*(G*x), not lv-scaled
    fck = np.ascontiguousarray(
        np.concatenate([fck_half, fck_half], axis=0)).astype(bf16)
    maps = []
    for ci in range(NCORES):
        dp2 = dp_all[ci]                                  # (32, ND) b0;b1
        dpm = np.concatenate([dp2[:, :1], dp2[:, :-1]], axis=1)   # dp[j-1]
        dpp = np.concatenate([dp2[:, 1:], dp2[:, -1:]], axis=1)   # dp[j+1]
        dpq = np.ascontiguousarray(
            np.concatenate([dp2, dpm, dp2, dpp], axis=0)).astype(bf16)
        maps.append({"xp": np.ascontiguousarray(xp_all[ci]),
                     "cph": cph, "dpq": dpq,
                     "cb": cb, "fck": fck, "nid": nid})
    return maps


def gather_out(results):
    out_full = np.empty((16, 128, N), np.float32)
    for ci in range(NCORES):
        o = np.asarray(results[ci]["out"], dtype=np.float32)
        o = np.transpose(o, (0, 2, 1, 3)).reshape(128, 2, N)
        out_full[2 * ci] = o[:, 0]
        out_full[2 * ci + 1] = o[:, 1]
    return out_full


_CACHED = {}


def _get_nc():
    if "nc" not in _CACHED:
        _CACHED["nc"] = build_nc()
    return _CACHED["nc"]


def kernel(deep, x, conv_w, conv_b, fc_w):
    in_maps = prep_inputs(deep, x, conv_w, conv_b, fc_w)
    nc = _get_nc()
    res = run_bass_kernel_spmd(nc, in_maps, core_ids=list(range(NCORES)))
    return gather_out(res.results)
